# revision 55
# baseline (speedup 1.0000x reference)
"""ASP layer (low-rank masked attention + residual layernorm) on 8 TRN2 cores.

Sharding: core c handles batch b = c // 2, query half h = c % 2.
Each core receives x/mask for its batch ROTATED so that its 1024 queries are
rows 0:1024 (keys are just permuted; softmax and delta are invariant to key
order). The device program is identical on all cores (SPMD); only data
differs.

Device math per core (N=2048 keys, Q=1024 queries, D=1024, R=64):
  QtKt = [U|V]^T @ x^T          (PE bf16, fp32 accum; x^T precomputed on host)
  Qt   = QtKt[0:64]   * (mask*s).T   (DVE; s = 1/sqrt(r_eff) folded on host)
  Kt   = QtKt[64:128] * mask.T       (DVE)
  St   = Kt_tile^T @ Qt         (PE; scores TRANSPOSED [k, q] so exp output
                                 is directly the delta stationary — no PE
                                 transposes of A needed.)
  Et   = exp(St - 3.5)          (ACT, psum -> sbuf FP8 e4m3. The -3.5 shift
                                 keeps exp <= ~100 < 240 (e4m3 max); softmax
                                 and the rs-scaled LN are shift-invariant.)
  rs   = ones^T @ Et            (PE fp8 DoubleRow; softmax row sums)
  delta= Et^T @ x8              (PE fp8 DoubleRow: 2 key-tiles contracted per
                                 instruction at 2x throughput; fp32 accum)
  z    = rs*x_q + delta         (DVE; LayerNorm is scale-invariant per token,
                                 so LN(x + delta/rs) = LN(rs*x + delta))
  out  = LN(z)                  (DVE bn_stats; single batched sqrt on ACT to
                                 avoid Exp<->Sqrt activation-table thrash)
gamma/beta are applied on the host (they are tiny per-feature vectors).
fp8 is used only where the softmax averaging washes the quantization out
(Et and the delta-side x); scores/projections stay bf16 — measured rel err
~9.6e-3 for this mix vs 1.9e-2 with fp8 scores (gate is 2e-2).
"""

import sys

sys.path.insert(0, "/opt/trn_rl_repo")

import numpy as np
import ml_dtypes

B, N, D, R = 4, 2048, 1024, 64
NCORES = 8
Q = N // 2          # queries per core
NQB = Q // 128      # query blocks per core
NKT = N // 128      # key tiles
NDT = D // 128      # d tiles
LN_EPS = 1e-5
WARMUP_MM = 82      # PE spin to lift the HAM clock gate early; sized to end
                    # just as the first x^T chunk lands in SBUF (ending early
                    # drops the P-state again and slows the whole stream)
EXP_SHIFT = -3.5    # exp(s + EXP_SHIFT): keeps Et below the e4m3 max of 240

BF16 = ml_dtypes.bfloat16
FP8 = ml_dtypes.float8_e4m3

_CACHE = {}
TRIM_TAIL = False


def _split_waits(nc, max_waits=1):
    """walrus in this container rejects instructions carrying more than ~1
    sem-wait (e.g. Drain/CTRL and the XPOSE DMA encodings). Move excess waits
    onto injected same-engine nops that precede the instruction — engines are
    program-ordered, so semantics are unchanged."""
    from concourse import mybir

    n = 0
    for fn in nc.m.functions:
        for bb in fn.blocks:
            insts = bb.instructions
            new_list = []
            for inst in insts:
                si = inst.sync_info
                waits = list(si.on_wait) if si and si.on_wait else []
                if len(waits) > max_waits:
                    excess = waits[: -max_waits]
                    si.on_wait = waits[-max_waits:]
                    for w in excess:
                        nop = mybir.InstNoOp(name=f"I-wsplit-{n}", ins=[],
                                             outs=[])
                        n += 1
                        nop.engine = inst.engine
                        nop.sync_info = mybir.SyncInfo(on_wait=[w],
                                                       on_update=[])
                        nc.register_instruction(nop)
                        new_list.append(nop)
                new_list.append(inst)
            insts[:] = new_list


def _patch_tile_drain():
    import concourse.tile as tile
    from concourse.vector_clock import ScopedClock

    if getattr(tile.TileContext, "_drain_patched", False):
        return

    def _drain_and_barrier(self, tick_clock, wait_clock):
        nc = self.nc
        drain_inst = nc.sync.drain()
        wait_clock.add_sem_waits(
            drain_inst.ins, ScopedClock({None: tick_clock.global_clock})
        )
        nc.all_engine_barrier()
        assert self.sems is not None
        popped = nc._tile_sem_poison_stack.pop()
        assert popped is self._sem_poison
        if not TRIM_TAIL:
            nc.clear_and_free_semaphores(list(self.sems.allocated().values()))
            nc.all_engine_barrier()
        _split_waits(nc)

    tile.TileContext._drain_and_barrier = _drain_and_barrier
    tile.TileContext._drain_patched = True


def build_program():
    import contextlib

    import concourse.bass as bass
    import concourse.tile as tile
    from concourse import mybir
    from concourse.tile import add_dep_helper

    _patch_tile_drain()
    f32 = mybir.dt.float32
    bf16 = mybir.dt.bfloat16
    fp8 = mybir.dt.float8e4
    AF = mybir.ActivationFunctionType
    DR = mybir.MatmulPerfMode.DoubleRow

    nc = bass.Bass("TRN2", target_bir_lowering=False, debug=False,
                   num_devices=NCORES)

    # x8/xt are stored PARTITION-MAJOR in dram (host pre-shuffle): each
    # partition's dram bytes are one contiguous run, so a fused x^T chunk
    # moves with 2KB DMA lines and an x8 quad with 4KB lines instead of 1KB
    # (descriptor-rate-bound DMA runs ~2x faster per queue at 2-4KB lines).
    x8_d = nc.dram_tensor("x8", [128, NKT, D], fp8, kind="ExternalInput").ap()
    xt_d = nc.dram_tensor("xt", [128, NDT, N], bf16,
                          kind="ExternalInput").ap()
    xq_d = nc.dram_tensor("xq", [Q, D], bf16, kind="ExternalInput").ap()
    mt_d = nc.dram_tensor("mt", [2 * R, N], bf16, kind="ExternalInput").ap()
    # uv pre-transposed on host to [128, NDT*2R] so the load is one clean
    # 2KB-per-partition DMA (the old "(t p) m -> p t m" rearrange generated
    # ~1024 tiny descriptors that sat in front of the critical x^T loads)
    uv_d = nc.dram_tensor("uv", [128, NDT, 2 * R], bf16,
                          kind="ExternalInput").ap()
    id_d = nc.dram_tensor("ident", [128, 128], f32, kind="ExternalInput").ap()
    out_d = nc.dram_tensor("out", [Q, D], bf16, kind="ExternalOutput").ap()

    with tile.TileContext(nc) as tc:
        with contextlib.ExitStack() as ctx:
            const = ctx.enter_context(tc.tile_pool(name="const", bufs=1))
            eps_sb = const.tile([128, 1], f32)
            nc.vector.memset(eps_sb, LN_EPS)
            shift_sb = const.tile([128, 1], f32)
            nc.vector.memset(shift_sb, EXP_SHIFT)
            ones_sb = const.tile([128, 2, 128], fp8)
            nc.vector.memset(ones_sb, 1.0)
            warm_sb = const.tile([128, 128], bf16)
            nc.vector.memset(warm_sb, 0.5)

            # Critical-path loads first: the first projection matmul needs
            # uv + the 8 x^T tiles of chunk 0; everything else queues behind.
            uv_sb = const.tile([128, NDT, 2 * R], bf16)
            xt_sb = const.tile([128, NDT, N], bf16)
            mt_sb = const.tile([2 * R, N], bf16)
            # uv/mt lead their rings (first proj matmul + first DVE mask-mul
            # need them); then x^T as fused 256KB half-row chunks (2KB
            # lines): wave 1 = cols 0:1024 (chunks 0+1), wave 2 = cols
            # 1024:2048 (chunks 2+3), alternating HWDGE rings (SP / ACT —
            # ACT is idle here, first exp is ~20us away).
            nc.sync.dma_start(out=uv_sb, in_=uv_d)
            nc.scalar.dma_start(out=mt_sb, in_=mt_d)
            # third ring: gpsimd's software-DGE queue (~110-170GB/s, idle
            # otherwise) carries d-tiles 6/7 of each wave, lifting load-phase
            # aggregate toward the ~350GB/s HBM cap
            xt_ring = [nc.sync, nc.scalar, nc.sync, nc.scalar,
                       nc.sync, nc.scalar, nc.gpsimd, nc.gpsimd]
            for half in range(2):
                lo, hi = half * 1024, (half + 1) * 1024
                for dt in range(NDT):
                    xt_ring[dt].dma_start(out=xt_sb[:, dt, lo:hi],
                                          in_=xt_d[:, dt, lo:hi])
            # ident feeds only the rs_fix transposes (~40us in)
            id_sb = const.tile([128, 128], f32)
            nc.gpsimd.dma_start(out=id_sb, in_=id_d)

            # PSUM bank budget is 8. Phase 0/A (scores live): st(4) + rr(1)
            # + d(3) = 8. Phase B (scores done): d(6) + rr(1) = 7, so the
            # delta stream holds 3 query blocks in flight and never waits on
            # the DVE epilogue to release a bank.
            P = {}
            P["rr"] = ctx.enter_context(
                tc.tile_pool(name="rr_ps", bufs=1, space="PSUM"))
            phaseA = ctx.enter_context(contextlib.ExitStack())
            st_pool = phaseA.enter_context(
                tc.tile_pool(name="st_ps", bufs=2, space="PSUM"))
            xq_pool = ctx.enter_context(tc.tile_pool(name="xq", bufs=2))

            # PE warm-up spin with NORMAL-mode matmuls (transpose-mode is
            # invisible to the HAM activity monitor): keeps TensorE busy so
            # the clock gate opens before the real matmul stream begins.
            with tc.tile_pool(name="warm", bufs=1, space="PSUM") as warm:
                w_ps = warm.tile([128, 128], f32)
                for _ in range(WARMUP_MM):
                    nc.tensor.matmul(w_ps, warm_sb, warm_sb,
                                     start=True, stop=True)
            # x8 is not needed until the first delta (~30us in): hold its
            # DMAs behind the late projection matmuls so the critical x^T
            # loads get the HBM bandwidth to themselves first
            x8_sb = const.tile([128, NKT, D], fp8)
            x8_dmas = []
            x8_ring = [nc.sync, nc.gpsimd, nc.sync, nc.gpsimd]
            for q in range(4):      # 512KB quads, 4KB lines, two rings
                x8_dmas.append(
                    x8_ring[q].dma_start(out=x8_sb[:, 4 * q:4 * q + 4, :],
                                         in_=x8_d[:, 4 * q:4 * q + 4, :]))

            qt_sb = const.tile([R, Q], bf16)
            kt_sb = const.tile([R, N], bf16)
            # Et layout: [p, qc, t(=kt pair), h, 512] — h indexes the kt pair
            # so [:, qc, t] is a ready-made [128, 2, 512] DoubleRow operand
            et_sb = const.tile([128, 2, NKT // 2, 2, 512], fp8)
            # throwaway Square output (only its accum_out matters); same-
            # engine WAW ordering makes sharing one buffer safe
            sq_scr = const.tile([128, 512], f32)
            work = ctx.enter_context(tc.tile_pool(name="work", bufs=2))
            keep = ctx.enter_context(tc.tile_pool(name="keep", bufs=1))
            small = ctx.enter_context(tc.tile_pool(name="small", bufs=3))
            rsq_sb = keep.tile([128, NQB], f32)   # softmax rowsums, [q,1]/qb


            def st_pair(qc, t):
                """St = Kt_kt^T @ Qt_qc for kt pair (2t, 2t+1); Et = exp."""
                qlo = qc * 512
                st_ps = st_pool.tile([128, 2, 512], f32, name=f"st_{qc}_{t}", tag="st")
                for h in range(2):
                    kt = 2 * t + h
                    nc.tensor.matmul(
                        st_ps[:, h],
                        kt_sb[:, kt * 128:(kt + 1) * 128],
                        qt_sb[:, qlo:qlo + 512],
                        start=True, stop=True,
                    )
                nc.scalar.activation(out=et_sb[:, qc, t], in_=st_ps,
                                     func=AF.Exp, bias=shift_sb)

            def rs_mm(qc, t, rr_ps):
                """one accumulating DoubleRow step of rs = ones^T @ Et; each
                step consumes a whole kt pair. The all-ones stationary is
                [128, 2, 128] (M=1 fails the walrus ldweights ISA check), so
                every psum partition receives the same rowsum row — rs_fix
                reads row 0."""
                nc.tensor.matmul(
                    rr_ps, ones_sb,
                    et_sb[:, qc, t],
                    start=(t == 0), stop=(t == NKT // 2 - 1),
                    perf_mode=DR,
                )

            def rs_copy(rr_ps):
                """rowsums psum -> sbuf; issued immediately after the last
                rs_mm so it lands ahead of the epilogue DVE backlog."""
                rs_sb = small.tile([1, 512], f32, tag="rs_sb")
                nc.vector.tensor_copy(rs_sb, rr_ps[0:1, :])
                return rs_sb

            def rs_fix(qc, rr_ps, rs_sb):
                """layout fix [1,q] -> [q,1] per query block via a tiny PE
                transpose; the shared PSUM bank hosts both outputs."""
                for j in range(4):
                    qb = qc * 4 + j
                    nc.tensor.transpose(rr_ps[:, j:j + 1],
                                        rs_sb[0:1, j * 128:(j + 1) * 128],
                                        id_sb[0:1, 0:1])
                    nc.vector.tensor_copy(rsq_sb[:, qb:qb + 1],
                                          rr_ps[:, j:j + 1])

            def delta_mms(qb):
                """Unnormalized delta = Et^T @ x8 (fp8 DoubleRow: one matmul
                contracts a whole kt pair), as a list of thunks so the caller
                can interleave them with other PE work. After each dc half's
                last matmul, its epilogue half (DVE) is issued inline."""
                qc, j = divmod(qb, 4)
                xq_sb = xq_pool.tile([128, D], bf16, name=f"xq_{qb}", tag="xq")
                xq_dma = nc.sync.dma_start(out=xq_sb,
                                           in_=xq_d[qb * 128:(qb + 1) * 128, :])
                if qb < 2:
                    add_dep_helper(xq_dma.ins, proj_last[1].ins,
                                   reason="delay early xq loads")
                d_ps = [P["d"].tile([128, 512], f32, name=f"d_{qb}_{dc}",
                                     tag="d") for dc in range(2)]
                y = work.tile([128, D], f32, tag="y")
                zs = small.tile([128, 2], f32, tag="zs")
                zss = small.tile([128, 2], f32, tag="zss")
                thunks = []
                for dc in range(2):
                    for t in range(NKT // 2):
                        def mm(dc=dc, t=t):
                            nc.tensor.matmul(
                                d_ps[dc],
                                et_sb[:, qc, t, :, j * 128:(j + 1) * 128],
                                x8_sb[:, 2 * t:2 * t + 2,
                                      dc * 512:(dc + 1) * 512],
                                start=(t == 0), stop=(t == NKT // 2 - 1),
                                perf_mode=DR,
                            )
                            if t == NKT // 2 - 1:
                                epi_half(qb, xq_sb, d_ps, dc, y, zs, zss)
                        thunks.append(mm)
                return y, (zs, zss), thunks

            def epi_half(qb, xq_sb, d_ps, dc, y, zs, zss):
                """z half: rs*x_q + delta for 512 features. sum(z) rides the
                same DVE op via accum_out; sum(z^2) goes to the idle ACT as
                Square+accum. Keeps bn_stats/bn_aggr off the busy DVE so it
                never falls behind the PE delta stream."""
                lo, hi = dc * 512, (dc + 1) * 512
                nc.vector.scalar_tensor_tensor(
                    out=y[:, lo:hi], in0=xq_sb[:, lo:hi],
                    scalar=rsq_sb[:, qb:qb + 1], in1=d_ps[dc],
                    op0=mybir.AluOpType.mult, op1=mybir.AluOpType.add,
                    accum_out=zs[:, dc:dc + 1],
                )
                nc.scalar.activation(out=sq_scr, in_=y[:, lo:hi],
                                     func=AF.Square,
                                     accum_out=zss[:, dc:dc + 1])

            def epi_finish(qb, y, zs, zss):
                """out = LN(z) from the accumulated moments:
                var = (sum(z^2) - sum(z)^2/D) / D; out = z*rstd - mean*rstd.
                Split per dc half so each half's store DMA starts early."""
                t1 = small.tile([128, 1], f32, tag="t1")
                nc.vector.tensor_add(t1, zs[:, 0:1], zs[:, 1:2])
                dv = small.tile([128, 1], f32, tag="dv")
                # dv = sum(z^2) - D*mean^2, via (t1/D)*t1 then subtract
                nc.vector.scalar_tensor_tensor(
                    out=dv, in0=t1, scalar=1.0 / D, in1=t1,
                    op0=mybir.AluOpType.mult, op1=mybir.AluOpType.mult)
                t2 = small.tile([128, 1], f32, tag="t2")
                nc.vector.tensor_add(t2, zss[:, 0:1], zss[:, 1:2])
                vv = small.tile([128, 1], f32, tag="vv")
                nc.vector.tensor_sub(vv, t2, dv)
                sd = small.tile([128, 1], f32, tag="sd")
                nc.scalar.activation(out=sd, in_=vv, func=AF.Sqrt,
                                     scale=1.0 / D, bias=eps_sb)
                rstd = small.tile([128, 1], f32, tag="rstd")
                nc.vector.reciprocal(rstd, sd)
                mr = small.tile([128, 1], f32, tag="mr")
                nc.vector.scalar_tensor_tensor(
                    out=mr, in0=t1, scalar=1.0 / D, in1=rstd,
                    op0=mybir.AluOpType.mult, op1=mybir.AluOpType.mult)
                o_sb = work.tile([128, D], bf16, tag="o")
                # store halves on both HWDGE rings: the final blocks' four
                # half-stores otherwise serialize ~2.8us of issue+queue time
                # on SP right at the kernel tail (ACT is idle by then)
                for dc in range(2):
                    lo, hi = dc * 512, (dc + 1) * 512
                    nc.vector.tensor_scalar(out=o_sb[:, lo:hi],
                                            in0=y[:, lo:hi],
                                            scalar1=rstd, scalar2=mr,
                                            op0=mybir.AluOpType.mult,
                                            op1=mybir.AluOpType.subtract)
                    eng = nc.sync if dc == 0 else nc.scalar
                    eng.dma_start(
                        out=out_d[qb * 128:(qb + 1) * 128, lo:hi],
                        in_=o_sb[:, lo:hi])

            # ---- phase 0/A: projections interleaved with qc0 scores so
            # the PE stream stays dense while exps run on ACT ----
            proj_last = {}
            rr0 = P["rr"].tile([128, 512], f32, name="rr_0", tag="rr")
            with tc.tile_pool(name="ps0", bufs=2, space="PSUM") as ps0:
                for nch in range(4):
                    lo, hi = nch * 512, (nch + 1) * 512
                    qk_ps = ps0.tile([128, 512], f32)
                    for dt in range(NDT):
                        mm = nc.tensor.matmul(
                            qk_ps, uv_sb[:, dt, :],
                            xt_sb[:, dt, lo:hi],
                            start=(dt == 0), stop=(dt == NDT - 1),
                        )
                    proj_last[nch] = mm
                    if lo < Q:
                        nc.vector.tensor_mul(qt_sb[:, lo:hi],
                                             qk_ps[0:R, :], mt_sb[0:R, lo:hi])
                    nc.vector.tensor_mul(kt_sb[:, lo:hi],
                                         qk_ps[R:2 * R, :],
                                         mt_sb[R:2 * R, lo:hi])
                    if nch >= 1:
                        # Kt tiles 0..4*nch-1 and Qt[0:512] are ready
                        st_pair(0, 2 * (nch - 1))
                        st_pair(0, 2 * (nch - 1) + 1)
                    if nch >= 2:
                        for t in range(2 * (nch - 2), 2 * (nch - 1)):
                            rs_mm(0, t, rr0)
                st_pair(0, 6)
                st_pair(0, 7)
                for t in range(4, 8):
                    rs_mm(0, t, rr0)
            for q, dma in enumerate(x8_dmas):
                anchor = proj_last[2 if q < 2 else 3]
                add_dep_helper(dma.ins, anchor.ins,
                               reason="delay x8 load behind x^T loads")

            P["d"] = phaseA.enter_context(
                tc.tile_pool(name="d_ps", bufs=3, space="PSUM"))
            rs_fix(0, rr0, rs_copy(rr0))

            # ---- qc1 scores interleaved with qb0/qb1 deltas; qc1 rowsums
            # lag the score stream by one kt pair so the PE never waits on
            # the exp that feeds them ----
            y0, zz0, th0 = delta_mms(0)
            y1, zz1, th1 = delta_mms(1)
            th01 = th0 + th1
            rr1 = P["rr"].tile([128, 512], f32, name="rr_1", tag="rr")
            for t in range(NKT // 2):
                st_pair(1, t)
                if t > 0:
                    rs_mm(1, t - 1, rr1)
                for mm in th01[t * 4:(t + 1) * 4]:
                    mm()
            rs_mm(1, NKT // 2 - 1, rr1)
            rs1_sb = rs_copy(rr1)
            epi_finish(0, y0, *zz0)
            epi_finish(1, y1, *zz1)

            phaseA.close()
            P["d"] = ctx.enter_context(
                tc.tile_pool(name="d_ps_b", bufs=6, space="PSUM"))

            # Phase B runs query blocks in pairs with dc-interleaved matmul
            # order (A0 B0 A1 B1): every d_ps stop except the final one gets
            # ~16 matmuls of PE cover for its DVE epilogue, so only the last
            # block's finish chain sits on the tail.
            for qa in range(2, NQB, 2):
                ya, zza, tha = delta_mms(qa)
                yb, zzb, thb = delta_mms(qa + 1)
                for i, mm in enumerate(tha[0:8] + thb[0:8] + tha[8:16]):
                    mm()
                    if qa == 2 and i == 3:
                        # qc1 rowsum fix rides between delta matmuls so its
                        # PE transposes never stall the stream
                        rs_fix(1, rr1, rs1_sb)
                epi_finish(qa, ya, *zza)
                for mm in thb[8:16]:
                    mm()
                epi_finish(qa + 1, yb, *zzb)

    return nc


def prep_core_inputs(x, mask, U, V):
    """Per-core input dicts (host-side sharding/layout prep)."""
    # [D, 2R] -> [128, NDT, 2R]: partition-major so the device DMA is one
    # contiguous 2KB-per-partition read
    uv = np.ascontiguousarray(
        np.concatenate([U, V], axis=1).astype(BF16)
        .reshape(NDT, 128, 2 * R).transpose(1, 0, 2))
    ident = np.eye(128, dtype=np.float32)
    ins = []
    for c in range(NCORES):
        b, h = divmod(c, 2)
        rot = np.roll(np.arange(N), -h * Q)
        xr = np.ascontiguousarray(x[b][rot])            # [N, D] f32
        mr = np.ascontiguousarray(mask[b][rot])         # [N, R] f32
        s = 1.0 / np.sqrt(np.maximum(mr.sum(axis=1), 1.0))   # [N]
        mq = (mr * s[:, None]).astype(BF16).T           # [R, N]
        mk = mr.astype(BF16).T                          # [R, N]
        xbf = xr.astype(BF16)
        ins.append({
            # partition-major: [tiles*128, free] -> [128, tiles, free] so
            # each partition's dram bytes are one contiguous run
            "x8": np.ascontiguousarray(
                xr.astype(FP8).reshape(NKT, 128, D).transpose(1, 0, 2)),
            "xt": np.ascontiguousarray(
                xbf.T.reshape(NDT, 128, N).transpose(1, 0, 2)),
            "xq": xbf[:Q],
            "mt": np.ascontiguousarray(np.concatenate([mq, mk], axis=0)),
            "uv": uv,
            "ident": ident,
        })
    return ins


def run_cores(ins, trace=False, trace_kwargs=None):
    from concourse.bass_utils import run_bass_kernel_spmd

    if "nc" not in _CACHE:
        _CACHE["nc"] = build_program()
    kw = {}
    if trace:
        kw["trace"] = True
        kw.update(trace_kwargs or {})
    return run_bass_kernel_spmd(_CACHE["nc"], ins, list(range(NCORES)), **kw)


def kernel(x, mask, U, V, gamma, beta):
    x = np.asarray(x, dtype=np.float32)
    mask = np.asarray(mask, dtype=np.float32)
    U = np.asarray(U, dtype=np.float32)
    V = np.asarray(V, dtype=np.float32)
    gamma = np.asarray(gamma, dtype=np.float32)
    beta = np.asarray(beta, dtype=np.float32)

    ins = prep_core_inputs(x, mask, U, V)
    res = run_cores(ins)
    out = np.empty((B, N, D), dtype=np.float32)
    for c in range(NCORES):
        b, h = divmod(c, 2)
        out[b, h * Q:(h + 1) * Q] = res.results[c]["out"].astype(np.float32)
    return out * gamma + beta



# revision 56
# speedup vs baseline: 1.0118x; 1.0118x over previous
"""ASP layer (low-rank masked attention + residual layernorm) on 8 TRN2 cores.

Sharding: core c handles batch b = c // 2, query half h = c % 2.
Each core receives x/mask for its batch ROTATED so that its 1024 queries are
rows 0:1024 (keys are just permuted; softmax and delta are invariant to key
order). The device program is identical on all cores (SPMD); only data
differs.

Device math per core (N=2048 keys, Q=1024 queries, D=1024, R=64):
  QtKt = [U|V]^T @ x^T          (PE bf16, fp32 accum; x^T precomputed on host)
  Qt   = QtKt[0:64]   * (mask*s).T   (DVE; s = 1/sqrt(r_eff) folded on host)
  Kt   = QtKt[64:128] * mask.T       (DVE)
  St   = Kt_tile^T @ Qt         (PE; scores TRANSPOSED [k, q] so exp output
                                 is directly the delta stationary — no PE
                                 transposes of A needed.)
  Et   = exp(St - 3.5)          (ACT, psum -> sbuf FP8 e4m3. The -3.5 shift
                                 keeps exp <= ~100 < 240 (e4m3 max); softmax
                                 and the rs-scaled LN are shift-invariant.)
  rs   = ones^T @ Et            (PE fp8 DoubleRow; softmax row sums)
  delta= Et^T @ x8              (PE fp8 DoubleRow: 2 key-tiles contracted per
                                 instruction at 2x throughput; fp32 accum)
  z    = rs*x_q + delta         (DVE; LayerNorm is scale-invariant per token,
                                 so LN(x + delta/rs) = LN(rs*x + delta))
  out  = LN(z)                  (DVE bn_stats; single batched sqrt on ACT to
                                 avoid Exp<->Sqrt activation-table thrash)
gamma/beta are applied on the host (they are tiny per-feature vectors).
fp8 is used only where the softmax averaging washes the quantization out
(Et and the delta-side x); scores/projections stay bf16 — measured rel err
~9.6e-3 for this mix vs 1.9e-2 with fp8 scores (gate is 2e-2).
"""

import sys

sys.path.insert(0, "/opt/trn_rl_repo")

import numpy as np
import ml_dtypes

B, N, D, R = 4, 2048, 1024, 64
NCORES = 8
Q = N // 2          # queries per core
NQB = Q // 128      # query blocks per core
NKT = N // 128      # key tiles
NDT = D // 128      # d tiles
LN_EPS = 1e-5
WARMUP_MM = 82      # PE spin to lift the HAM clock gate early; sized to end
                    # just as the first x^T chunk lands in SBUF (ending early
                    # drops the P-state again and slows the whole stream)
EXP_SHIFT = -3.5    # exp(s + EXP_SHIFT): keeps Et below the e4m3 max of 240

BF16 = ml_dtypes.bfloat16
FP8 = ml_dtypes.float8_e4m3

_CACHE = {}
TRIM_TAIL = False


def _split_waits(nc, max_waits=1):
    """walrus in this container rejects instructions carrying more than ~1
    sem-wait (e.g. Drain/CTRL and the XPOSE DMA encodings). Move excess waits
    onto injected same-engine nops that precede the instruction — engines are
    program-ordered, so semantics are unchanged."""
    from concourse import mybir

    n = 0
    for fn in nc.m.functions:
        for bb in fn.blocks:
            insts = bb.instructions
            new_list = []
            for inst in insts:
                si = inst.sync_info
                waits = list(si.on_wait) if si and si.on_wait else []
                if len(waits) > max_waits:
                    excess = waits[: -max_waits]
                    si.on_wait = waits[-max_waits:]
                    for w in excess:
                        nop = mybir.InstNoOp(name=f"I-wsplit-{n}", ins=[],
                                             outs=[])
                        n += 1
                        nop.engine = inst.engine
                        nop.sync_info = mybir.SyncInfo(on_wait=[w],
                                                       on_update=[])
                        nc.register_instruction(nop)
                        new_list.append(nop)
                new_list.append(inst)
            insts[:] = new_list


def _patch_tile_drain():
    import concourse.tile as tile
    from concourse.vector_clock import ScopedClock

    if getattr(tile.TileContext, "_drain_patched", False):
        return

    def _drain_and_barrier(self, tick_clock, wait_clock):
        nc = self.nc
        drain_inst = nc.sync.drain()
        wait_clock.add_sem_waits(
            drain_inst.ins, ScopedClock({None: tick_clock.global_clock})
        )
        nc.all_engine_barrier()
        assert self.sems is not None
        popped = nc._tile_sem_poison_stack.pop()
        assert popped is self._sem_poison
        if not TRIM_TAIL:
            nc.clear_and_free_semaphores(list(self.sems.allocated().values()))
            nc.all_engine_barrier()
        _split_waits(nc)

    tile.TileContext._drain_and_barrier = _drain_and_barrier
    tile.TileContext._drain_patched = True


def build_program():
    import contextlib

    import concourse.bass as bass
    import concourse.tile as tile
    from concourse import mybir
    from concourse.tile import add_dep_helper

    _patch_tile_drain()
    f32 = mybir.dt.float32
    bf16 = mybir.dt.bfloat16
    fp8 = mybir.dt.float8e4
    AF = mybir.ActivationFunctionType
    DR = mybir.MatmulPerfMode.DoubleRow

    nc = bass.Bass("TRN2", target_bir_lowering=False, debug=False,
                   num_devices=NCORES)

    # x8/xt are stored PARTITION-MAJOR in dram (host pre-shuffle): each
    # partition's dram bytes are one contiguous run, so a fused x^T chunk
    # moves with 2KB DMA lines and an x8 quad with 4KB lines instead of 1KB
    # (descriptor-rate-bound DMA runs ~2x faster per queue at 2-4KB lines).
    x8_d = nc.dram_tensor("x8", [128, NKT, D], fp8, kind="ExternalInput").ap()
    xt_d = nc.dram_tensor("xt", [128, NDT, N], bf16,
                          kind="ExternalInput").ap()
    xq_d = nc.dram_tensor("xq", [Q, D], bf16, kind="ExternalInput").ap()
    mt_d = nc.dram_tensor("mt", [2 * R, N], bf16, kind="ExternalInput").ap()
    # uv pre-transposed on host to [128, NDT*2R] so the load is one clean
    # 2KB-per-partition DMA (the old "(t p) m -> p t m" rearrange generated
    # ~1024 tiny descriptors that sat in front of the critical x^T loads)
    uv_d = nc.dram_tensor("uv", [128, NDT, 2 * R], bf16,
                          kind="ExternalInput").ap()
    id_d = nc.dram_tensor("ident", [128, 128], f32, kind="ExternalInput").ap()
    out_d = nc.dram_tensor("out", [Q, D], bf16, kind="ExternalOutput").ap()

    with tile.TileContext(nc) as tc:
        with contextlib.ExitStack() as ctx:
            const = ctx.enter_context(tc.tile_pool(name="const", bufs=1))
            eps_sb = const.tile([128, 1], f32)
            nc.vector.memset(eps_sb, LN_EPS)
            shift_sb = const.tile([128, 1], f32)
            nc.vector.memset(shift_sb, EXP_SHIFT)
            ones_sb = const.tile([128, 2, 128], fp8)
            nc.vector.memset(ones_sb, 1.0)
            warm_sb = const.tile([128, 128], bf16)
            nc.vector.memset(warm_sb, 0.5)

            # Critical-path loads first: the first projection matmul needs
            # uv + the 8 x^T tiles of chunk 0; everything else queues behind.
            uv_sb = const.tile([128, NDT, 2 * R], bf16)
            xt_sb = const.tile([128, NDT, N], bf16)
            mt_sb = const.tile([2 * R, N], bf16)
            # uv/mt lead their rings (first proj matmul + first DVE mask-mul
            # need them); then x^T as fused 256KB half-row chunks (2KB
            # lines): wave 1 = cols 0:1024 (chunks 0+1), wave 2 = cols
            # 1024:2048 (chunks 2+3), alternating HWDGE rings (SP / ACT —
            # ACT is idle here, first exp is ~20us away).
            nc.sync.dma_start(out=uv_sb, in_=uv_d)
            nc.scalar.dma_start(out=mt_sb, in_=mt_d)
            for half in range(2):
                lo, hi = half * 1024, (half + 1) * 1024
                for dt in range(NDT):
                    eng = nc.sync if dt % 2 == 0 else nc.scalar
                    eng.dma_start(out=xt_sb[:, dt, lo:hi],
                                  in_=xt_d[:, dt, lo:hi])
            # ident feeds only the rs_fix transposes (~40us in)
            id_sb = const.tile([128, 128], f32)
            nc.sync.dma_start(out=id_sb, in_=id_d)

            # PSUM bank budget is 8. Phase 0/A (scores live): st(4) + rr(1)
            # + d(3) = 8. Phase B (scores done): d(6) + rr(1) = 7, so the
            # delta stream holds 3 query blocks in flight and never waits on
            # the DVE epilogue to release a bank.
            P = {}
            P["rr"] = ctx.enter_context(
                tc.tile_pool(name="rr_ps", bufs=1, space="PSUM"))
            phaseA = ctx.enter_context(contextlib.ExitStack())
            st_pool = phaseA.enter_context(
                tc.tile_pool(name="st_ps", bufs=2, space="PSUM"))
            xq_pool = ctx.enter_context(tc.tile_pool(name="xq", bufs=2))

            # PE warm-up spin with NORMAL-mode matmuls (transpose-mode is
            # invisible to the HAM activity monitor): keeps TensorE busy so
            # the clock gate opens before the real matmul stream begins.
            with tc.tile_pool(name="warm", bufs=1, space="PSUM") as warm:
                w_ps = warm.tile([128, 128], f32)
                for _ in range(WARMUP_MM):
                    nc.tensor.matmul(w_ps, warm_sb, warm_sb,
                                     start=True, stop=True)
            # x8 is not needed until the first delta (~30us in): hold its
            # DMAs behind the late projection matmuls so the critical x^T
            # loads get the HBM bandwidth to themselves first
            x8_sb = const.tile([128, NKT, D], fp8)
            x8_dmas = []
            # quads split across the sync ring and the otherwise-idle
            # gpsimd software ring so x8 finishes ~2x sooner; the x^T
            # loads keep the two fast HWDGE rings to themselves
            x8_ring = [nc.sync, nc.gpsimd, nc.sync, nc.gpsimd]
            for q in range(4):      # 512KB quads, 4KB lines
                x8_dmas.append(
                    x8_ring[q].dma_start(out=x8_sb[:, 4 * q:4 * q + 4, :],
                                         in_=x8_d[:, 4 * q:4 * q + 4, :]))

            qt_sb = const.tile([R, Q], bf16)
            kt_sb = const.tile([R, N], bf16)
            # Et layout: [p, qc, t(=kt pair), h, 512] — h indexes the kt pair
            # so [:, qc, t] is a ready-made [128, 2, 512] DoubleRow operand
            et_sb = const.tile([128, 2, NKT // 2, 2, 512], fp8)
            # throwaway Square output (only its accum_out matters); same-
            # engine WAW ordering makes sharing one buffer safe
            sq_scr = const.tile([128, 512], f32)
            work = ctx.enter_context(tc.tile_pool(name="work", bufs=2))
            keep = ctx.enter_context(tc.tile_pool(name="keep", bufs=1))
            small = ctx.enter_context(tc.tile_pool(name="small", bufs=3))
            rsq_sb = keep.tile([128, NQB], f32)   # softmax rowsums, [q,1]/qb


            def st_pair(qc, t):
                """St = Kt_kt^T @ Qt_qc for kt pair (2t, 2t+1); Et = exp."""
                qlo = qc * 512
                st_ps = st_pool.tile([128, 2, 512], f32, name=f"st_{qc}_{t}", tag="st")
                for h in range(2):
                    kt = 2 * t + h
                    nc.tensor.matmul(
                        st_ps[:, h],
                        kt_sb[:, kt * 128:(kt + 1) * 128],
                        qt_sb[:, qlo:qlo + 512],
                        start=True, stop=True,
                    )
                nc.scalar.activation(out=et_sb[:, qc, t], in_=st_ps,
                                     func=AF.Exp, bias=shift_sb)

            def rs_mm(qc, t, rr_ps):
                """one accumulating DoubleRow step of rs = ones^T @ Et; each
                step consumes a whole kt pair. The all-ones stationary is
                [128, 2, 128] (M=1 fails the walrus ldweights ISA check), so
                every psum partition receives the same rowsum row — rs_fix
                reads row 0."""
                nc.tensor.matmul(
                    rr_ps, ones_sb,
                    et_sb[:, qc, t],
                    start=(t == 0), stop=(t == NKT // 2 - 1),
                    perf_mode=DR,
                )

            def rs_copy(rr_ps):
                """rowsums psum -> sbuf; issued immediately after the last
                rs_mm so it lands ahead of the epilogue DVE backlog."""
                rs_sb = small.tile([1, 512], f32, tag="rs_sb")
                nc.vector.tensor_copy(rs_sb, rr_ps[0:1, :])
                return rs_sb

            def rs_fix(qc, rr_ps, rs_sb):
                """layout fix [1,q] -> [q,1] per query block via a tiny PE
                transpose; the shared PSUM bank hosts both outputs."""
                for j in range(4):
                    qb = qc * 4 + j
                    nc.tensor.transpose(rr_ps[:, j:j + 1],
                                        rs_sb[0:1, j * 128:(j + 1) * 128],
                                        id_sb[0:1, 0:1])
                    nc.vector.tensor_copy(rsq_sb[:, qb:qb + 1],
                                          rr_ps[:, j:j + 1])

            def delta_mms(qb):
                """Unnormalized delta = Et^T @ x8 (fp8 DoubleRow: one matmul
                contracts a whole kt pair), as a list of thunks so the caller
                can interleave them with other PE work. After each dc half's
                last matmul, its epilogue half (DVE) is issued inline."""
                qc, j = divmod(qb, 4)
                xq_sb = xq_pool.tile([128, D], bf16, name=f"xq_{qb}", tag="xq")
                xq_dma = nc.sync.dma_start(out=xq_sb,
                                           in_=xq_d[qb * 128:(qb + 1) * 128, :])
                if qb < 2:
                    add_dep_helper(xq_dma.ins, proj_last[1].ins,
                                   reason="delay early xq loads")
                d_ps = [P["d"].tile([128, 512], f32, name=f"d_{qb}_{dc}",
                                     tag="d") for dc in range(2)]
                y = work.tile([128, D], f32, tag="y")
                zs = small.tile([128, 2], f32, tag="zs")
                zss = small.tile([128, 2], f32, tag="zss")
                thunks = []
                for dc in range(2):
                    for t in range(NKT // 2):
                        def mm(dc=dc, t=t):
                            nc.tensor.matmul(
                                d_ps[dc],
                                et_sb[:, qc, t, :, j * 128:(j + 1) * 128],
                                x8_sb[:, 2 * t:2 * t + 2,
                                      dc * 512:(dc + 1) * 512],
                                start=(t == 0), stop=(t == NKT // 2 - 1),
                                perf_mode=DR,
                            )
                            if t == NKT // 2 - 1:
                                epi_half(qb, xq_sb, d_ps, dc, y, zs, zss)
                        thunks.append(mm)
                return y, (zs, zss), thunks

            def epi_half(qb, xq_sb, d_ps, dc, y, zs, zss):
                """z half: rs*x_q + delta for 512 features. sum(z) rides the
                same DVE op via accum_out; sum(z^2) goes to the idle ACT as
                Square+accum. Keeps bn_stats/bn_aggr off the busy DVE so it
                never falls behind the PE delta stream."""
                lo, hi = dc * 512, (dc + 1) * 512
                nc.vector.scalar_tensor_tensor(
                    out=y[:, lo:hi], in0=xq_sb[:, lo:hi],
                    scalar=rsq_sb[:, qb:qb + 1], in1=d_ps[dc],
                    op0=mybir.AluOpType.mult, op1=mybir.AluOpType.add,
                    accum_out=zs[:, dc:dc + 1],
                )
                nc.scalar.activation(out=sq_scr, in_=y[:, lo:hi],
                                     func=AF.Square,
                                     accum_out=zss[:, dc:dc + 1])

            def epi_finish(qb, y, zs, zss):
                """out = LN(z) from the accumulated moments:
                var = (sum(z^2) - sum(z)^2/D) / D; out = z*rstd - mean*rstd.
                Split per dc half so each half's store DMA starts early."""
                t1 = small.tile([128, 1], f32, tag="t1")
                nc.vector.tensor_add(t1, zs[:, 0:1], zs[:, 1:2])
                dv = small.tile([128, 1], f32, tag="dv")
                # dv = sum(z^2) - D*mean^2, via (t1/D)*t1 then subtract
                nc.vector.scalar_tensor_tensor(
                    out=dv, in0=t1, scalar=1.0 / D, in1=t1,
                    op0=mybir.AluOpType.mult, op1=mybir.AluOpType.mult)
                t2 = small.tile([128, 1], f32, tag="t2")
                nc.vector.tensor_add(t2, zss[:, 0:1], zss[:, 1:2])
                vv = small.tile([128, 1], f32, tag="vv")
                nc.vector.tensor_sub(vv, t2, dv)
                sd = small.tile([128, 1], f32, tag="sd")
                nc.scalar.activation(out=sd, in_=vv, func=AF.Sqrt,
                                     scale=1.0 / D, bias=eps_sb)
                rstd = small.tile([128, 1], f32, tag="rstd")
                nc.vector.reciprocal(rstd, sd)
                mr = small.tile([128, 1], f32, tag="mr")
                nc.vector.scalar_tensor_tensor(
                    out=mr, in0=t1, scalar=1.0 / D, in1=rstd,
                    op0=mybir.AluOpType.mult, op1=mybir.AluOpType.mult)
                o_sb = work.tile([128, D], bf16, tag="o")
                # store halves on both HWDGE rings: the final blocks' four
                # half-stores otherwise serialize ~2.8us of issue+queue time
                # on SP right at the kernel tail (ACT is idle by then)
                for dc in range(2):
                    lo, hi = dc * 512, (dc + 1) * 512
                    nc.vector.tensor_scalar(out=o_sb[:, lo:hi],
                                            in0=y[:, lo:hi],
                                            scalar1=rstd, scalar2=mr,
                                            op0=mybir.AluOpType.mult,
                                            op1=mybir.AluOpType.subtract)
                    eng = nc.sync if dc == 0 else nc.scalar
                    eng.dma_start(
                        out=out_d[qb * 128:(qb + 1) * 128, lo:hi],
                        in_=o_sb[:, lo:hi])

            # ---- phase 0/A: projections interleaved with qc0 scores so
            # the PE stream stays dense while exps run on ACT ----
            proj_last = {}
            rr0 = P["rr"].tile([128, 512], f32, name="rr_0", tag="rr")
            with tc.tile_pool(name="ps0", bufs=2, space="PSUM") as ps0:
                for nch in range(4):
                    lo, hi = nch * 512, (nch + 1) * 512
                    qk_ps = ps0.tile([128, 512], f32)
                    for dt in range(NDT):
                        mm = nc.tensor.matmul(
                            qk_ps, uv_sb[:, dt, :],
                            xt_sb[:, dt, lo:hi],
                            start=(dt == 0), stop=(dt == NDT - 1),
                        )
                    proj_last[nch] = mm
                    if lo < Q:
                        nc.vector.tensor_mul(qt_sb[:, lo:hi],
                                             qk_ps[0:R, :], mt_sb[0:R, lo:hi])
                    nc.vector.tensor_mul(kt_sb[:, lo:hi],
                                         qk_ps[R:2 * R, :],
                                         mt_sb[R:2 * R, lo:hi])
                    if nch >= 1:
                        # Kt tiles 0..4*nch-1 and Qt[0:512] are ready
                        st_pair(0, 2 * (nch - 1))
                        st_pair(0, 2 * (nch - 1) + 1)
                    if nch >= 2:
                        for t in range(2 * (nch - 2), 2 * (nch - 1)):
                            rs_mm(0, t, rr0)
                st_pair(0, 6)
                st_pair(0, 7)
                for t in range(4, 8):
                    rs_mm(0, t, rr0)
            for q, dma in enumerate(x8_dmas):
                anchor = proj_last[2 if q < 2 else 3]
                add_dep_helper(dma.ins, anchor.ins,
                               reason="delay x8 load behind x^T loads")

            P["d"] = phaseA.enter_context(
                tc.tile_pool(name="d_ps", bufs=3, space="PSUM"))
            rs_fix(0, rr0, rs_copy(rr0))

            # ---- qc1 scores interleaved with qb0/qb1 deltas; qc1 rowsums
            # lag the score stream by one kt pair so the PE never waits on
            # the exp that feeds them ----
            y0, zz0, th0 = delta_mms(0)
            y1, zz1, th1 = delta_mms(1)
            th01 = th0 + th1
            rr1 = P["rr"].tile([128, 512], f32, name="rr_1", tag="rr")
            for t in range(NKT // 2):
                st_pair(1, t)
                if t > 0:
                    rs_mm(1, t - 1, rr1)
                for mm in th01[t * 4:(t + 1) * 4]:
                    mm()
            rs_mm(1, NKT // 2 - 1, rr1)
            rs1_sb = rs_copy(rr1)
            epi_finish(0, y0, *zz0)
            epi_finish(1, y1, *zz1)

            phaseA.close()
            P["d"] = ctx.enter_context(
                tc.tile_pool(name="d_ps_b", bufs=6, space="PSUM"))

            # Phase B runs query blocks in pairs with dc-interleaved matmul
            # order (A0 B0 A1 B1): every d_ps stop except the final one gets
            # ~16 matmuls of PE cover for its DVE epilogue, so only the last
            # block's finish chain sits on the tail.
            for qa in range(2, NQB, 2):
                ya, zza, tha = delta_mms(qa)
                yb, zzb, thb = delta_mms(qa + 1)
                for i, mm in enumerate(tha[0:8] + thb[0:8] + tha[8:16]):
                    mm()
                    if qa == 2 and i == 3:
                        # qc1 rowsum fix rides between delta matmuls so its
                        # PE transposes never stall the stream
                        rs_fix(1, rr1, rs1_sb)
                epi_finish(qa, ya, *zza)
                for mm in thb[8:16]:
                    mm()
                epi_finish(qa + 1, yb, *zzb)

    return nc


def prep_core_inputs(x, mask, U, V):
    """Per-core input dicts (host-side sharding/layout prep)."""
    # [D, 2R] -> [128, NDT, 2R]: partition-major so the device DMA is one
    # contiguous 2KB-per-partition read
    uv = np.ascontiguousarray(
        np.concatenate([U, V], axis=1).astype(BF16)
        .reshape(NDT, 128, 2 * R).transpose(1, 0, 2))
    ident = np.eye(128, dtype=np.float32)
    ins = []
    for c in range(NCORES):
        b, h = divmod(c, 2)
        rot = np.roll(np.arange(N), -h * Q)
        xr = np.ascontiguousarray(x[b][rot])            # [N, D] f32
        mr = np.ascontiguousarray(mask[b][rot])         # [N, R] f32
        s = 1.0 / np.sqrt(np.maximum(mr.sum(axis=1), 1.0))   # [N]
        mq = (mr * s[:, None]).astype(BF16).T           # [R, N]
        mk = mr.astype(BF16).T                          # [R, N]
        xbf = xr.astype(BF16)
        ins.append({
            # partition-major: [tiles*128, free] -> [128, tiles, free] so
            # each partition's dram bytes are one contiguous run
            "x8": np.ascontiguousarray(
                xr.astype(FP8).reshape(NKT, 128, D).transpose(1, 0, 2)),
            "xt": np.ascontiguousarray(
                xbf.T.reshape(NDT, 128, N).transpose(1, 0, 2)),
            "xq": xbf[:Q],
            "mt": np.ascontiguousarray(np.concatenate([mq, mk], axis=0)),
            "uv": uv,
            "ident": ident,
        })
    return ins


def run_cores(ins, trace=False, trace_kwargs=None):
    from concourse.bass_utils import run_bass_kernel_spmd

    if "nc" not in _CACHE:
        _CACHE["nc"] = build_program()
    kw = {}
    if trace:
        kw["trace"] = True
        kw.update(trace_kwargs or {})
    return run_bass_kernel_spmd(_CACHE["nc"], ins, list(range(NCORES)), **kw)


def kernel(x, mask, U, V, gamma, beta):
    x = np.asarray(x, dtype=np.float32)
    mask = np.asarray(mask, dtype=np.float32)
    U = np.asarray(U, dtype=np.float32)
    V = np.asarray(V, dtype=np.float32)
    gamma = np.asarray(gamma, dtype=np.float32)
    beta = np.asarray(beta, dtype=np.float32)

    ins = prep_core_inputs(x, mask, U, V)
    res = run_cores(ins)
    out = np.empty((B, N, D), dtype=np.float32)
    for c in range(NCORES):
        b, h = divmod(c, 2)
        out[b, h * Q:(h + 1) * Q] = res.results[c]["out"].astype(np.float32)
    return out * gamma + beta

